# revision 1
# baseline (speedup 1.0000x reference)
"""Bass/Tile kernel builder for sharded MultiHeadAttention on TRN2.

Sharding: 8 cores = 2 batches x 4 head-groups (4 heads each, e-slice of 256).
Each core computes a partial output outT [1024, 2048] (bf16, transposed);
host sums the 4 head-group partials per batch and transposes back.

Schedule (vs the original single-rep structure):
  - weights/biases/ones loaded ONCE per NEFF (outside the rep loop)
  - x tensors for rep r+1 prefetched mid-rep-r (gpsimd dispatch queue;
    stores stay on sync) -> no Sync head-of-line stall at rep boundary
  - K/Q projections split into two et-passes over a 4-bank psK pool:
    first QK+exp starts after half the proj work; next rep's K-proj only
    needs 4 free PSUM banks so it overlaps this rep's oproj tail
  - kt/qt double-buffered across reps

Device dataflow (per core):
  QT[e,t] = wqT.T @ xqT   (scale 1/8 folded into wqT/bq on host)
  KT[e,t] = wkT.T @ xkT
  V[t,e]  = xvT.T @ wvT   (per t-block with a ones column per head)
  per head:  S^T[j,i] = KT_h.T-block @ QT_h   (K=64, PSUM f32)
             P^T = exp(S^T)                   (ACT, bf16, no max subtraction)
             O^T[65,i] += V_aug.T @ P^T       (row 64 = softmax denominator)
             A_h = O^T[0:64] * reciprocal(denom)
  outT[f,t] = woT.T @ A + bo
"""
from contextlib import ExitStack

import concourse.bass as bass
import concourse.tile as tile
from concourse import bacc, mybir

F32 = mybir.dt.float32
BF16 = mybir.dt.bfloat16

T = 2048          # sequence length
D = 1024          # d_model
E = 256           # per-core projection width (4 heads x 64)
HPC = 4           # heads per core
DH = 64           # head dim
KB = D // 128     # contraction blocks for projections
TB = T // 128     # t-blocks / j-blocks
IC = 1024         # attention i-chunk
NIC = T // IC
PRE_BUFS = 24     # P-tile slots


class _Bacc(bacc.Bacc):
    # Keep matmul waits on the MATMUL so LDWEIGHTS stays wait-free and can
    # pre-load during the previous matmul (excess waits become EVSEMs).
    def move_matmul_waits_to_ldweights(self):
        pass


def build_nc(reps=1):
    nc = _Bacc("TRN2", target_bir_lowering=False, debug=False,
               enable_asserts=False, num_devices=8)
    din = {}
    for name in ("xqT", "xkT", "xvT"):
        din[name] = nc.dram_tensor(name, [D, T], BF16, kind="ExternalInput").ap()
    for name in ("wqT", "wkT", "wvT"):
        din[name] = nc.dram_tensor(name, [D, E], BF16, kind="ExternalInput").ap()
    din["woT"] = nc.dram_tensor("woT", [E, D], BF16, kind="ExternalInput").ap()
    din["bq"] = nc.dram_tensor("bq", [E], F32, kind="ExternalInput").ap()
    din["bk"] = nc.dram_tensor("bk", [E], F32, kind="ExternalInput").ap()
    din["bv"] = nc.dram_tensor("bv", [E], F32, kind="ExternalInput").ap()
    din["bo"] = nc.dram_tensor("bo", [D], F32, kind="ExternalInput").ap()
    outT = nc.dram_tensor("outT", [D, T], BF16, kind="ExternalOutput").ap()

    with tile.TileContext(nc) as tc:
        _build(tc, nc, din, outT, reps)
    nc.compile()
    return nc


def _build(tc, nc, din, outT, reps):
    with ExitStack() as ctx:
        per = ctx.enter_context(tc.tile_pool(name="per", bufs=1))

        # ---- persistent SBUF tensors: weights/biases/ones, loaded once ----
        wq = per.tile([128, KB, E], BF16, tag="wq")
        wk = per.tile([128, KB, E], BF16, tag="wk")
        wv = per.tile([128, KB, E], BF16, tag="wv")
        wo = per.tile([128, 2, D], BF16, tag="wo")
        bq = per.tile([128, 2], F32, tag="bq")
        bk = per.tile([128, 2], F32, tag="bk")
        bvb = per.tile([128, E], F32, tag="bvb")
        bo = per.tile([128, KB], F32, tag="bo")
        vv = per.tile([128, TB, HPC * (DH + 1)], BF16, tag="vv")
        aa = per.tile([128, 2, T], BF16, tag="aa")    # normalized attn out A^T

        nc.sync.dma_start(bq[:], din["bq"].rearrange("(a p) -> p a", p=128))
        nc.sync.dma_start(bk[:], din["bk"].rearrange("(a p) -> p a", p=128))
        nc.sync.dma_start(bo[:], din["bo"].rearrange("(a p) -> p a", p=128))
        nc.sync.dma_start(bvb[:], din["bv"].partition_broadcast(128))
        # ones columns for the denominator trick (V columns rewritten per rep)
        nc.vector.memset(vv[:], 1.0)

        # PE clock warmup: dummy accumulating matmuls on the ones tile while
        # the first x chunks are still in flight (HAM ramps on busy time)
        with tc.tile_pool(name="psW", bufs=1, space="PSUM") as psW:
            wrm = psW.tile([128, 512], F32, tag="W")
            for i in range(40):
                nc.tensor.matmul(wrm[:, 0:256], vv[:, 0, 0:128],
                                 vv[:, 1, 0:256],
                                 start=(i == 0), stop=(i == 39))

        # double-buffered per-rep tensors
        sbKQ = ctx.enter_context(tc.tile_pool(name="sbKQ", bufs=2))
        sbX = ctx.enter_context(tc.tile_pool(name="sbX", bufs=2))

        def alloc_x(r):
            """Allocate + DMA the rep-r input tiles."""
            xk = sbX.tile([128, KB, T], BF16, tag="x", name=f"xk{r}")
            xq = sbX.tile([128, KB, T], BF16, tag="x", name=f"xq{r}")
            xv = sbX.tile([128, KB, T], BF16, tag="x", name=f"xv{r}")
            for xname, xdst in (("xkT", xk), ("xqT", xq), ("xvT", xv)):
                src = din[xname].rearrange("(kb p) t -> p kb t", p=128)
                for kb in range(KB):
                    nc.sync.dma_start(xdst[:, kb, :], src[:, kb, :])
            return xk, xq, xv

        # rep-0 inputs, ordered so the K-path lands first: wk (chunked)
        # then xk, then the Q-path, then V, with wo last
        xk0 = sbX.tile([128, KB, T], BF16, tag="x", name="xk0")
        xq0 = sbX.tile([128, KB, T], BF16, tag="x", name="xq0")
        xv0 = sbX.tile([128, KB, T], BF16, tag="x", name="xv0")
        wsrc = {n: din[n].rearrange("(kb p) t -> p kb t", p=128)
                for n in ("wkT", "wqT", "wvT", "woT")}
        xsrc = {n: din[n].rearrange("(kb p) t -> p kb t", p=128)
                for n in ("xkT", "xqT", "xvT")}
        for kb in range(KB):
            nc.sync.dma_start(wk[:, kb, :], wsrc["wkT"][:, kb, :])
        for kb in range(KB):
            nc.sync.dma_start(xk0[:, kb, :], xsrc["xkT"][:, kb, :])
        nc.sync.dma_start(wq[:], wsrc["wqT"])
        for kb in range(KB):
            nc.sync.dma_start(xq0[:, kb, :], xsrc["xqT"][:, kb, :])
        nc.sync.dma_start(wv[:], wsrc["wvT"])
        for kb in range(KB):
            nc.sync.dma_start(xv0[:, kb, :], xsrc["xvT"][:, kb, :])
        nc.sync.dma_start(wo[:], wsrc["woT"])

        xt = (xk0, xq0, xv0)
        for r in range(reps):
            nxt = [None]

            def prefetch(r=r):
                if r + 1 < reps:
                    nxt[0] = alloc_x(r + 1)

            _rep(tc, nc, din, outT, r,
                 wq, wk, wv, wo, bq, bk, bvb, bo, vv, aa, sbKQ,
                 xt, prefetch)
            xt = nxt[0]


def _rep(tc, nc, din, outT, r,
         wq, wk, wv, wo, bq, bk, bvb, bo, vv, aa, sbKQ, xt, prefetch):
    Exp = mybir.ActivationFunctionType.Exp
    Ident = mybir.ActivationFunctionType.Identity
    xk, xq, xv = xt

    kt = sbKQ.tile([128, 2, T], BF16, tag="kt", name=f"kt{r}")
    qt = sbKQ.tile([128, 2, T], BF16, tag="qt", name=f"qt{r}")

    with ExitStack() as ctx:
        actx = ctx.enter_context(ExitStack())
        psS = actx.enter_context(
            tc.tile_pool(name=f"psS{r}", bufs=2, space="PSUM"))
        sbP = actx.enter_context(tc.tile_pool(name=f"sbP{r}", bufs=PRE_BUFS))
        sbN = actx.enter_context(tc.tile_pool(name=f"sbN{r}", bufs=3))

        def qk_exp(h, ic, jb):
            et, eo = h // 2, (h % 2) * 64
            st = psS.tile([128, IC], F32, tag="S", name=f"st{h}_{ic}_{jb}_{r}")
            for nn in range(IC // 512):
                nc.tensor.matmul(
                    st[:, nn * 512:(nn + 1) * 512],
                    kt[eo:eo + DH, et, jb * 128:(jb + 1) * 128],
                    qt[eo:eo + DH, et,
                       ic * IC + nn * 512: ic * IC + (nn + 1) * 512],
                    start=True, stop=True)
            pt = sbP.tile([128, IC], BF16, tag="P", name=f"pt{h}_{ic}_{jb}_{r}")
            nc.scalar.activation(pt[:], st[:], Exp)
            return pt

        with tc.tile_pool(name=f"psK{r}", bufs=4, space="PSUM") as psK:

            def proj_pass(pname, w_t, b_t, src, dst, et):
                """One et-pass of a K/Q projection: 4 psum groups x 8 kb."""
                pss = [psK.tile([128, 512], F32, tag="pp",
                                name=f"pp_{pname}{et}_{g}_{r}") for g in range(4)]
                for kb in range(KB):
                    for nch in range(4):
                        nc.tensor.matmul(
                            pss[nch][:],
                            w_t[:, kb, et * 128:(et + 1) * 128],
                            src[:, kb, nch * 512:(nch + 1) * 512],
                            start=(kb == 0), stop=(kb == KB - 1))
                for nch in range(4):
                    nc.vector.tensor_scalar_add(
                        dst[:, et, nch * 512:(nch + 1) * 512],
                        pss[nch][:], b_t[:, et:et + 1])

            # et0 passes, then the prefetched h0 chunk's QK+exp (so the
            # PE reaches them before the et1 passes and ACT starts ~30us
            # earlier), then et1
            proj_pass("k", wk, bk, xk, kt, 0)
            proj_pass("q", wq, bq, xq, qt, 0)
            pts0 = [qk_exp(0, 0, jb) for jb in range(TB)]
            proj_pass("k", wk, bk, xk, kt, 1)
            proj_pass("q", wq, bq, xq, qt, 1)

        # ---- attention (+ V-proj interleaved through the O-tag slots) ----
        with tc.tile_pool(name=f"psO{r}", bufs=4, space="PSUM") as psO:

            def pv(h, oc, jb, pt):
                for nn in range(IC // 512):
                    nc.tensor.matmul(
                        oc[nn][0:DH + 1, :],
                        vv[:, jb, h * (DH + 1):(h + 1) * (DH + 1)],
                        pt[:, nn * 512:(nn + 1) * 512],
                        start=(jb == 0), stop=(jb == TB - 1))

            def alloc_oc(h, ic):
                return [psO.tile([128, 512], F32, tag="O",
                                 name=f"oc{h}_{ic}_{i}_{r}")
                        for i in range(IC // 512)]

            def normalize(h, ic, oc):
                et, eo = h // 2, (h % 2) * 64
                for nn in range(IC // 512):
                    base = ic * IC + nn * 512
                    ops = oc[nn]
                    dcp = sbN.tile([1, 512], F32, tag="dcp")
                    nc.vector.tensor_copy(dcp[:], ops[DH:DH + 1, :])
                    rr = sbN.tile([1, 512], F32, tag="rr")
                    nc.vector.reciprocal_approx_fast(rr[:], dcp[:])
                    rb = sbN.tile([DH, 512], F32, tag="rb")
                    nc.gpsimd.partition_broadcast(rb[:], rr[:])
                    nc.vector.tensor_mul(aa[eo:eo + DH, et, base:base + 512],
                                         ops[0:DH, :], rb[:])

            # V projection through the O-tag slots: 4 waves of 4 t-blocks
            for wave in range(4):
                tbs = list(range(wave * 4, wave * 4 + 4))
                psv = [psO.tile([128, 512], F32, tag="O", name=f"vp_{tb}_{r}")
                       for tb in tbs]
                for kb in range(KB):
                    for i, tb in enumerate(tbs):
                        nc.tensor.matmul(
                            psv[i][:, 0:E],
                            xv[:, kb, tb * 128:(tb + 1) * 128],
                            wv[:, kb, :],
                            start=(kb == 0), stop=(kb == KB - 1))
                for i, tb in enumerate(tbs):
                    for h in range(HPC):
                        nc.vector.tensor_add(
                            vv[:, tb, h * (DH + 1): h * (DH + 1) + DH],
                            psv[i][:, h * DH:(h + 1) * DH],
                            bvb[:, h * DH:(h + 1) * DH])

            # prefetch next rep's inputs: emitted here so the scheduler
            # gives the dispatches mid-rep priority
            prefetch()

            # PV for the prefetched chunk
            oc = alloc_oc(0, 0)
            for jb in range(TB):
                pv(0, oc, jb, pts0[jb])
            normalize(0, 0, oc)

            # rest of the attention
            for h in range(HPC):
                for ic in range(NIC):
                    if h == 0 and ic == 0:
                        continue
                    oc = alloc_oc(h, ic)
                    for jb in range(TB):
                        pt = qk_exp(h, ic, jb)
                        pv(h, oc, jb, pt)
                    normalize(h, ic, oc)

        actx.close()  # free psS/sbP/sbN before the oproj staging pools

        # ---- output projection ----
        with tc.tile_pool(name=f"psC{r}", bufs=4, space="PSUM") as psC, \
             tc.tile_pool(name=f"sbO{r}", bufs=4) as sbO:
            for ft in range(KB):  # 8 f-blocks of 128
                stg = sbO.tile([128, T], BF16, tag="stg")
                for nch in range(4):  # t chunks of 512
                    ps = psC.tile([128, 512], F32, tag="op")
                    for kb in range(2):
                        nc.tensor.matmul(
                            ps[:],
                            wo[:, kb, ft * 128:(ft + 1) * 128],
                            aa[:, kb, nch * 512:(nch + 1) * 512],
                            start=(kb == 0), stop=(kb == 1))
                    # alternate PSUM->SBUF bias-add between DVE and ACT
                    dst = stg[:, nch * 512:(nch + 1) * 512]
                    if nch % 2 == 0:
                        nc.vector.tensor_scalar_add(dst, ps[:], bo[:, ft:ft + 1])
                    else:
                        nc.scalar.activation(dst, ps[:], Ident,
                                             bias=bo[:, ft:ft + 1])
                nc.sync.dma_start(
                    outT.rearrange("(ft p) t -> p ft t", p=128)[:, ft, :],
                    stg[:])


# ======================== host-side wrapper ========================
import numpy as np
import ml_dtypes

NP_BF16 = ml_dtypes.bfloat16
B = 2
NCORES = 8
GPB = 4
_CACHE = {}


def _core_inputs(c, q, k, v, Wq, bq, Wk, bk, Wv, bv, Wo, bo):
    b, g = divmod(c, GPB)
    es = slice(g * E, g * E + E)
    return {
        "xqT": np.ascontiguousarray(q[b].T).astype(NP_BF16),
        "xkT": np.ascontiguousarray(k[b].T).astype(NP_BF16),
        "xvT": np.ascontiguousarray(v[b].T).astype(NP_BF16),
        "wqT": np.ascontiguousarray((Wq[es, :] / 8.0).T).astype(NP_BF16),
        "wkT": np.ascontiguousarray(Wk[es, :].T).astype(NP_BF16),
        "wvT": np.ascontiguousarray(Wv[es, :].T).astype(NP_BF16),
        "woT": np.ascontiguousarray(Wo[:, es].T).astype(NP_BF16),
        "bq": (np.asarray(bq)[es] / 8.0).astype(np.float32),
        "bk": np.asarray(bk)[es].astype(np.float32),
        "bv": np.asarray(bv)[es].astype(np.float32),
        "bo": (np.asarray(bo) if g == 0 else np.zeros_like(bo)).astype(np.float32),
    }


def kernel(q, k, v, Wq, bq, Wk, bk, Wv, bv, Wo, bo):
    """Full-input MultiHeadAttention on 8 NeuronCores; returns [2,2048,1024] f32."""
    from concourse.bass_utils import run_bass_kernel_spmd

    if "nc" not in _CACHE:
        _CACHE["nc"] = build_nc()
    nc = _CACHE["nc"]

    args = dict(q=np.asarray(q, np.float32), k=np.asarray(k, np.float32),
                v=np.asarray(v, np.float32), Wq=np.asarray(Wq, np.float32),
                bq=np.asarray(bq, np.float32), Wk=np.asarray(Wk, np.float32),
                bk=np.asarray(bk, np.float32), Wv=np.asarray(Wv, np.float32),
                bv=np.asarray(bv, np.float32), Wo=np.asarray(Wo, np.float32),
                bo=np.asarray(bo, np.float32))
    in_maps = [_core_inputs(c, **args) for c in range(NCORES)]
    res = run_bass_kernel_spmd(nc, in_maps, core_ids=list(range(NCORES)))
    out = np.zeros((B, T, D), np.float32)
    for c, r in enumerate(res.results):
        out[c // GPB] += r["outT"].T.astype(np.float32)
    return out



# revision 7
# speedup vs baseline: 1.2839x; 1.2839x over previous
"""Bass/Tile kernel builder for sharded MultiHeadAttention on TRN2.

Sharding: 8 cores = 2 batches x 4 head-groups (4 heads each, e-slice of 256).
Each core computes a partial output outT [1024, 2048] (bf16, transposed);
host sums the 4 head-group partials per batch and transposes back.

Schedule highlights:
  - weights/biases/ones loaded ONCE per NEFF (outside the rep loop)
  - heads processed in PAIRS: head h%2==0 lives on SBUF partitions 0-63,
    h%2==1 on 64-127, so the pair's K=64 QK matmuls land on PE row-tiles
    (0,0)/(64,0) and stream CONCURRENTLY (2x QK throughput)
  - softmax exp split across engines per j-block: ~9/16 of tiles on ACT
    (exact exp) and ~7/16 on DVE via a Schraudolph bit-trick
    (int16(S*128*log2e + 16249.3) viewed as bf16 == 2^(S*log2e) to ~1.8%,
    zero-mean after softmax normalization)
  - V-proj runs through the psK pool during the projection phase
  - x tensors for rep r+1 prefetched mid-rep
  - per head: S^T[j,i] = KT_h.T-block @ QT_h (K=64, PSUM f32)
              P^T ~= exp(S^T)               (ACT or DVE, bf16)
              O^T[65,i] += V_aug.T @ P^T    (row 64 = softmax denominator)
              A_h = O^T[0:64] * reciprocal(denom)
  - outT[f,t] = woT.T @ A + bo
"""
from contextlib import ExitStack

import concourse.bass as bass
import concourse.tile as tile
from concourse import bacc, mybir

F32 = mybir.dt.float32
BF16 = mybir.dt.bfloat16
I16 = mybir.dt.int16

T = 2048          # sequence length
D = 1024          # d_model
E = 256           # per-core projection width (4 heads x 64)
HPC = 4           # heads per core
DH = 64           # head dim
KB = D // 128     # contraction blocks for projections
TB = T // 128     # t-blocks / j-blocks
IC = 1024         # attention i-chunk
NIC = T // IC
PRE_BUFS = 26     # P-tile slots
PRE_JB = 8        # j-blocks of pair 0 / ic 0 prefetched during proj phase

# Schraudolph-exp constants for bf16-viewed int16:
#   i16 = S*128*log2(e) + (127*128 - 6.7)  -> bitcast bf16 ~= exp(S)
SCH_A = 184.6649652337873
SCH_B = 16249.3
# j-blocks whose exp goes to DVE (even head / odd head of each pair)
DVE_E = frozenset((1, 3, 5, 7, 9, 11, 13))
DVE_O = frozenset((2, 4, 6, 8, 10, 12, 15))


class _Bacc(bacc.Bacc):
    # Keep matmul waits on the MATMUL so LDWEIGHTS stays wait-free and can
    # pre-load during the previous matmul (excess waits become EVSEMs).
    def move_matmul_waits_to_ldweights(self):
        pass


def build_nc(reps=1):
    nc = _Bacc("TRN2", target_bir_lowering=False, debug=False,
               enable_asserts=False, num_devices=8)
    din = {}
    for name in ("xqT", "xkT", "xvT"):
        din[name] = nc.dram_tensor(name, [D, T], BF16, kind="ExternalInput").ap()
    for name in ("wqT", "wkT", "wvT"):
        din[name] = nc.dram_tensor(name, [D, E], BF16, kind="ExternalInput").ap()
    din["woT"] = nc.dram_tensor("woT", [E, D], BF16, kind="ExternalInput").ap()
    din["bq"] = nc.dram_tensor("bq", [E], F32, kind="ExternalInput").ap()
    din["bk"] = nc.dram_tensor("bk", [E], F32, kind="ExternalInput").ap()
    din["bv"] = nc.dram_tensor("bv", [E], F32, kind="ExternalInput").ap()
    din["bo"] = nc.dram_tensor("bo", [D], F32, kind="ExternalInput").ap()
    outT = nc.dram_tensor("outT", [D, T], BF16, kind="ExternalOutput").ap()

    with tile.TileContext(nc) as tc:
        _build(tc, nc, din, outT, reps)
    nc.compile()
    return nc


def _build(tc, nc, din, outT, reps):
    with ExitStack() as ctx:
        per = ctx.enter_context(tc.tile_pool(name="per", bufs=1))

        # ---- persistent SBUF tensors: weights/biases/ones, loaded once ----
        wq = per.tile([128, KB, E], BF16, tag="wq")
        wk = per.tile([128, KB, E], BF16, tag="wk")
        wv = per.tile([128, KB, E], BF16, tag="wv")
        wo = per.tile([128, 2, D], BF16, tag="wo")
        bq = per.tile([128, 2], F32, tag="bq")
        bk = per.tile([128, 2], F32, tag="bk")
        bvb = per.tile([128, E], F32, tag="bvb")
        bo = per.tile([128, KB], F32, tag="bo")
        vv = per.tile([128, TB, HPC * (DH + 1)], BF16, tag="vv")

        nc.sync.dma_start(bq[:], din["bq"].rearrange("(a p) -> p a", p=128))
        nc.sync.dma_start(bk[:], din["bk"].rearrange("(a p) -> p a", p=128))
        nc.sync.dma_start(bo[:], din["bo"].rearrange("(a p) -> p a", p=128))
        nc.sync.dma_start(bvb[:], din["bv"].partition_broadcast(128))
        # ones columns for the denominator trick (V columns rewritten per rep)
        nc.vector.memset(vv[:], 1.0)

        # PE clock warmup: dummy accumulating matmuls on the ones tile while
        # the first x chunks are still in flight (HAM ramps on busy time)
        with tc.tile_pool(name="psW", bufs=1, space="PSUM") as psW:
            wrm = psW.tile([128, 512], F32, tag="W")
            for i in range(40):
                nc.tensor.matmul(wrm[:, 0:256], vv[:, 0, 0:128],
                                 vv[:, 1, 0:256],
                                 start=(i == 0), stop=(i == 39))

        # double-buffered per-rep tensors
        sbKQ = ctx.enter_context(tc.tile_pool(name="sbKQ", bufs=2))
        sbA = ctx.enter_context(tc.tile_pool(name="sbA", bufs=2))
        sbX = ctx.enter_context(tc.tile_pool(name="sbX", bufs=2))

        def alloc_x(r):
            """Allocate + DMA the rep-r input tiles."""
            xk = sbX.tile([128, KB, T], BF16, tag="x", name=f"xk{r}")
            xq = sbX.tile([128, KB, T], BF16, tag="x", name=f"xq{r}")
            xv = sbX.tile([128, KB, T], BF16, tag="x", name=f"xv{r}")
            for xname, xdst in (("xkT", xk), ("xqT", xq), ("xvT", xv)):
                src = din[xname].rearrange("(kb p) t -> p kb t", p=128)
                for kb in range(KB):
                    nc.sync.dma_start(xdst[:, kb, :], src[:, kb, :])
            return xk, xq, xv

        # rep-0 inputs, ordered so the K-path lands first
        xk0 = sbX.tile([128, KB, T], BF16, tag="x", name="xk0")
        xq0 = sbX.tile([128, KB, T], BF16, tag="x", name="xq0")
        xv0 = sbX.tile([128, KB, T], BF16, tag="x", name="xv0")
        wsrc = {n: din[n].rearrange("(kb p) t -> p kb t", p=128)
                for n in ("wkT", "wqT", "wvT", "woT")}
        xsrc = {n: din[n].rearrange("(kb p) t -> p kb t", p=128)
                for n in ("xkT", "xqT", "xvT")}
        for kb in range(KB):
            nc.sync.dma_start(wk[:, kb, :], wsrc["wkT"][:, kb, :])
        for kb in range(KB):
            nc.sync.dma_start(xk0[:, kb, :], xsrc["xkT"][:, kb, :])
        nc.sync.dma_start(wq[:], wsrc["wqT"])
        for kb in range(KB):
            nc.sync.dma_start(xq0[:, kb, :], xsrc["xqT"][:, kb, :])
        nc.sync.dma_start(wv[:], wsrc["wvT"])
        for kb in range(KB):
            nc.sync.dma_start(xv0[:, kb, :], xsrc["xvT"][:, kb, :])
        nc.sync.dma_start(wo[:], wsrc["woT"])

        xt = (xk0, xq0, xv0)
        for r in range(reps):
            nxt = [None]

            def prefetch(r=r):
                if r + 1 < reps:
                    nxt[0] = alloc_x(r + 1)

            _rep(tc, nc, din, outT, r,
                 wq, wk, wv, wo, bq, bk, bvb, bo, vv, sbKQ, sbA,
                 xt, prefetch)
            xt = nxt[0]


def _rep(tc, nc, din, outT, r,
         wq, wk, wv, wo, bq, bk, bvb, bo, vv, sbKQ, sbA, xt, prefetch):
    Exp = mybir.ActivationFunctionType.Exp
    Ident = mybir.ActivationFunctionType.Identity
    xk, xq, xv = xt

    kt = sbKQ.tile([128, 2, T], BF16, tag="kt", name=f"kt{r}")
    qt = sbKQ.tile([128, 2, T], BF16, tag="qt", name=f"qt{r}")
    aa = sbA.tile([128, 2, T], BF16, tag="aa", name=f"aa{r}")

    with ExitStack() as ctx:
        actx = ctx.enter_context(ExitStack())
        psS = actx.enter_context(
            tc.tile_pool(name=f"psS{r}", bufs=2, space="PSUM"))
        sbP = actx.enter_context(tc.tile_pool(name=f"sbP{r}", bufs=PRE_BUFS))
        sbN = actx.enter_context(tc.tile_pool(name=f"sbN{r}", bufs=2))

        def qk_exp(h, ic, jb, dve):
            et, eo = h // 2, (h % 2) * 64
            st = psS.tile([128, IC], F32, tag="S", name=f"st{h}_{ic}_{jb}_{r}")
            for nn in range(IC // 512):
                nc.tensor.matmul(
                    st[:, nn * 512:(nn + 1) * 512],
                    kt[eo:eo + DH, et, jb * 128:(jb + 1) * 128],
                    qt[eo:eo + DH, et,
                       ic * IC + nn * 512: ic * IC + (nn + 1) * 512],
                    start=True, stop=True)
            if dve:
                pt = sbP.tile([128, IC], I16, tag="P",
                              name=f"pt{h}_{ic}_{jb}_{r}")
                nc.vector.tensor_scalar(
                    pt[:], st[:], SCH_A, SCH_B,
                    mybir.AluOpType.mult, mybir.AluOpType.add)
                return pt[:].bitcast(BF16)
            pt = sbP.tile([128, IC], BF16, tag="P", name=f"pt{h}_{ic}_{jb}_{r}")
            nc.scalar.activation(pt[:], st[:], Exp)
            return pt[:]

        with tc.tile_pool(name=f"psK{r}", bufs=4, space="PSUM") as psK:

            def proj_pass(pname, w_t, b_t, src, dst, et):
                """One et-pass of a K/Q projection: 4 psum groups x 8 kb."""
                pss = [psK.tile([128, 512], F32, tag="pp",
                                name=f"pp_{pname}{et}_{g}_{r}") for g in range(4)]
                for kb in range(KB):
                    for nch in range(4):
                        nc.tensor.matmul(
                            pss[nch][:],
                            w_t[:, kb, et * 128:(et + 1) * 128],
                            src[:, kb, nch * 512:(nch + 1) * 512],
                            start=(kb == 0), stop=(kb == KB - 1))
                for nch in range(4):
                    nc.vector.tensor_scalar_add(
                        dst[:, et, nch * 512:(nch + 1) * 512],
                        pss[nch][:], b_t[:, et:et + 1])

            # et0 passes (heads 0,1), then the prefetched pair-0 QK+exp
            # chunks (feeds ACT/DVE during the remaining proj work), then et1
            proj_pass("k", wk, bk, xk, kt, 0)
            proj_pass("q", wq, bq, xq, qt, 0)
            early = []
            for jb in range(PRE_JB):
                pe_ = qk_exp(0, 0, jb, dve=(jb in DVE_E))
                po_ = qk_exp(1, 0, jb, dve=(jb in DVE_O))
                early.append((pe_, po_))
            proj_pass("k", wk, bk, xk, kt, 1)
            proj_pass("q", wq, bq, xq, qt, 1)

            # V projection through the psK slots: 4 waves of 4 t-blocks
            bvb4 = bvb[:, :].rearrange("p (h d) -> p h d", h=HPC)
            for wave in range(4):
                tbs = list(range(wave * 4, wave * 4 + 4))
                psv = [psK.tile([128, 512], F32, tag="pp", name=f"vp_{tb}_{r}")
                       for tb in tbs]
                for kb in range(KB):
                    for i, tb in enumerate(tbs):
                        nc.tensor.matmul(
                            psv[i][:, 0:E],
                            xv[:, kb, tb * 128:(tb + 1) * 128],
                            wv[:, kb, :],
                            start=(kb == 0), stop=(kb == KB - 1))
                for i, tb in enumerate(tbs):
                    dstv = vv[:, tb, :].rearrange(
                        "p (h x) -> p h x", h=HPC)[:, :, 0:DH]
                    srcv = psv[i][:, 0:E].rearrange("p (h d) -> p h d", h=HPC)
                    nc.vector.tensor_add(dstv, srcv, bvb4)

        # ---- attention ----
        with tc.tile_pool(name=f"psO{r}", bufs=2, space="PSUM") as psO:

            def pv(h, oc, jb, pt):
                for nn in range(IC // 512):
                    nc.tensor.matmul(
                        oc[0:DH + 1, nn * 512:(nn + 1) * 512],
                        vv[:, jb, h * (DH + 1):(h + 1) * (DH + 1)],
                        pt[:, nn * 512:(nn + 1) * 512],
                        start=(jb == 0), stop=(jb == TB - 1))

            def normalize(h, ic, oc):
                et, eo = h // 2, (h % 2) * 64
                for nn in range(IC // 512):
                    base = ic * IC + nn * 512
                    ops = oc[:, nn * 512:(nn + 1) * 512]
                    dcp = sbN.tile([1, 512], F32, tag="dcp")
                    nc.vector.tensor_copy(dcp[:], ops[DH:DH + 1, :])
                    rr = sbN.tile([1, 512], F32, tag="rr")
                    nc.vector.reciprocal_approx_fast(rr[:], dcp[:])
                    rb = sbN.tile([DH, 512], F32, tag="rb")
                    nc.gpsimd.partition_broadcast(rb[:], rr[:])
                    nc.vector.tensor_mul(aa[eo:eo + DH, et, base:base + 512],
                                         ops[0:DH, :], rb[:])

            # prefetch next rep's inputs: emitted here so the scheduler
            # gives the dispatches mid-rep priority
            prefetch()

            for pr in range(2):
                he, ho = 2 * pr, 2 * pr + 1
                for ic in range(NIC):
                    oc_e = psO.tile([128, IC], F32, tag="O",
                                    name=f"oc{he}_{ic}_{r}")
                    oc_o = psO.tile([128, IC], F32, tag="O",
                                    name=f"oc{ho}_{ic}_{r}")
                    for jb in range(TB):
                        if pr == 0 and ic == 0 and jb < PRE_JB:
                            pe_, po_ = early[jb]
                        else:
                            pe_ = qk_exp(he, ic, jb, dve=(jb in DVE_E))
                            po_ = qk_exp(ho, ic, jb, dve=(jb in DVE_O))
                        pv(he, oc_e, jb, pe_)
                        pv(ho, oc_o, jb, po_)
                    normalize(he, ic, oc_e)
                    normalize(ho, ic, oc_o)

        actx.close()  # free psS/sbP/sbN before the oproj staging pools

        # ---- output projection ----
        with tc.tile_pool(name=f"psC{r}", bufs=4, space="PSUM") as psC, \
             tc.tile_pool(name=f"sbO{r}", bufs=4) as sbO:
            for ft in range(KB):  # 8 f-blocks of 128
                stg = sbO.tile([128, T], BF16, tag="stg")
                for nch in range(4):  # t chunks of 512
                    ps = psC.tile([128, 512], F32, tag="op")
                    for kb in range(2):
                        nc.tensor.matmul(
                            ps[:],
                            wo[:, kb, ft * 128:(ft + 1) * 128],
                            aa[:, kb, nch * 512:(nch + 1) * 512],
                            start=(kb == 0), stop=(kb == 1))
                    # alternate PSUM->SBUF bias-add between DVE and ACT
                    dst = stg[:, nch * 512:(nch + 1) * 512]
                    if nch % 2 == 0:
                        nc.vector.tensor_scalar_add(dst, ps[:], bo[:, ft:ft + 1])
                    else:
                        nc.scalar.activation(dst, ps[:], Ident,
                                             bias=bo[:, ft:ft + 1])
                nc.sync.dma_start(
                    outT.rearrange("(ft p) t -> p ft t", p=128)[:, ft, :],
                    stg[:])


# ======================== host-side wrapper ========================
import numpy as np
import ml_dtypes

NP_BF16 = ml_dtypes.bfloat16
B = 2
NCORES = 8
GPB = 4
_CACHE = {}


def _core_inputs(c, q, k, v, Wq, bq, Wk, bk, Wv, bv, Wo, bo):
    b, g = divmod(c, GPB)
    es = slice(g * E, g * E + E)
    return {
        "xqT": np.ascontiguousarray(q[b].T).astype(NP_BF16),
        "xkT": np.ascontiguousarray(k[b].T).astype(NP_BF16),
        "xvT": np.ascontiguousarray(v[b].T).astype(NP_BF16),
        "wqT": np.ascontiguousarray((Wq[es, :] / 8.0).T).astype(NP_BF16),
        "wkT": np.ascontiguousarray(Wk[es, :].T).astype(NP_BF16),
        "wvT": np.ascontiguousarray(Wv[es, :].T).astype(NP_BF16),
        "woT": np.ascontiguousarray(Wo[:, es].T).astype(NP_BF16),
        "bq": (np.asarray(bq)[es] / 8.0).astype(np.float32),
        "bk": np.asarray(bk)[es].astype(np.float32),
        "bv": np.asarray(bv)[es].astype(np.float32),
        "bo": (np.asarray(bo) if g == 0 else np.zeros_like(bo)).astype(np.float32),
    }


def kernel(q, k, v, Wq, bq, Wk, bk, Wv, bv, Wo, bo):
    """Full-input MultiHeadAttention on 8 NeuronCores; returns [2,2048,1024] f32."""
    from concourse.bass_utils import run_bass_kernel_spmd

    if "nc" not in _CACHE:
        _CACHE["nc"] = build_nc()
    nc = _CACHE["nc"]

    args = dict(q=np.asarray(q, np.float32), k=np.asarray(k, np.float32),
                v=np.asarray(v, np.float32), Wq=np.asarray(Wq, np.float32),
                bq=np.asarray(bq, np.float32), Wk=np.asarray(Wk, np.float32),
                bk=np.asarray(bk, np.float32), Wv=np.asarray(Wv, np.float32),
                bv=np.asarray(bv, np.float32), Wo=np.asarray(Wo, np.float32),
                bo=np.asarray(bo, np.float32))
    in_maps = [_core_inputs(c, **args) for c in range(NCORES)]
    res = run_bass_kernel_spmd(nc, in_maps, core_ids=list(range(NCORES)))
    out = np.zeros((B, T, D), np.float32)
    for c, r in enumerate(res.results):
        out[c // GPB] += r["outT"].T.astype(np.float32)
    return out
